# revision 6
# baseline (speedup 1.0000x reference)
"""Trainium2 Bass kernel for nn_CausalSelfAttention (B=1, T=2048, D=1024, H=16).

Sharding: 2 heads per core across 8 cores (tensor parallel). Wq/Wk/Wv
column-sharded by head, attention fully local, Wo row-sharded; host sums the
8 partial outputs (the all-reduce of the unshard step).

Per-core pipeline (all matmuls fp32r):
  P1  fused QKV: psum[t,384] = sum_i xT_blk.T @ [WqT|WkT|(1-l)WvT]; RMS stats
      (Square+segmented reduce -> Ln -> Exp = rsqrt, 0.125 folded into k's
      scale), RoPE via concat-table trick (y = qk*ccat + swap(qk)*scat),
      v-blend (+lam*vi) fused into PSUM evacuation, [vA|1|vB|1] tiles.
  P2  PE-transpose roped q,k -> qT,kT (d-major, f32r).
  P3  per (ci, head): ST[tk,tq] = kT_slice.T @ qT_chunk into 2-bank PSUM duos,
      one Exp per duo, tri-mask on diagonal blocks, matmul2 YT[d|L,tq] with
      lhsT=[v|1] and rhs=E (no P transposes), e^sink accumulated via K=1 matmul
      so scale = sigmoid(lse-sink)/L = 1/(L + e^sink).
  P4  broadcast 1/(L+e^sink) across partitions via K=1 matmul + reciprocal,
      scale YT -> yTs_h, out-proj per head (K=64) accumulating, evacuate, DMA.
"""

import sys

if "/opt/trn_rl_repo" not in sys.path:
    sys.path.insert(0, "/opt/trn_rl_repo")

import numpy as np
from contextlib import ExitStack

from concourse import bacc, tile
from concourse import mybir
from concourse.bass_utils import run_bass_kernel_spmd

F32 = mybir.dt.float32
F32R = mybir.dt.float32r
AF = mybir.ActivationFunctionType
ALU = mybir.AluOpType
AX = mybir.AxisListType

T = 2048
D = 1024
HD = 64
NT = T // 128  # 16 t-tiles
RMS_EPS = float(np.finfo(np.float32).eps)
LN8 = float(np.log(0.125))


def _build_program():
    nc = bacc.Bacc("TRN2", target_bir_lowering=False, debug=False, num_devices=8)

    d_xT = nc.dram_tensor("xT", [D, T], F32R, kind="ExternalInput").ap()
    d_wqkv = nc.dram_tensor("wqkv", [D, 384], F32R, kind="ExternalInput").ap()
    d_vis = nc.dram_tensor("vis", [T, 128], F32, kind="ExternalInput").ap()
    d_cc = nc.dram_tensor("cc", [T, 64], F32, kind="ExternalInput").ap()
    d_sc = nc.dram_tensor("sc", [T, 64], F32, kind="ExternalInput").ap()
    d_wo0 = nc.dram_tensor("wo0", [64, D], F32R, kind="ExternalInput").ap()
    d_wo1 = nc.dram_tensor("wo1", [64, D], F32R, kind="ExternalInput").ap()
    d_idn = nc.dram_tensor("idn", [128, 128], F32R, kind="ExternalInput").ap()
    d_tri = nc.dram_tensor("tri", [128, 128], F32, kind="ExternalInput").ap()
    d_esk = nc.dram_tensor("esk", [1, 130], F32R, kind="ExternalInput").ap()
    d_on1 = nc.dram_tensor("on1", [1, 512], F32R, kind="ExternalInput").ap()
    d_onp = nc.dram_tensor("onp", [65, 64], F32R, kind="ExternalInput").ap()
    d_on2 = nc.dram_tensor("on2", [128, 2], F32, kind="ExternalInput").ap()
    d_out = nc.dram_tensor("out", [T, D], F32, kind="ExternalOutput").ap()

    with tile.TileContext(nc) as tc, ExitStack() as ctx:
        sb = ctx.enter_context(tc.tile_pool(name="sb", bufs=1))
        sb_x = ctx.enter_context(tc.tile_pool(name="sb_x", bufs=4))
        sb_w1 = ctx.enter_context(tc.tile_pool(name="sb_w1", bufs=3))
        sb_w2 = ctx.enter_context(tc.tile_pool(name="sb_w2", bufs=3))
        sb_e = ctx.enter_context(tc.tile_pool(name="sb_e", bufs=3))
        sb_o = ctx.enter_context(tc.tile_pool(name="sb_o", bufs=3))

        wqkv = sb.tile([128, 8, 384], F32R)
        nc.sync.dma_start(out=wqkv[:], in_=d_wqkv.rearrange("(n p) c -> p n c", p=128))
        vi_t = sb.tile([128, NT, 128], F32)
        nc.sync.dma_start(out=vi_t[:], in_=d_vis.rearrange("(n p) c -> p n c", p=128))
        cc_t = sb.tile([128, NT, 64], F32)
        nc.sync.dma_start(out=cc_t[:], in_=d_cc.rearrange("(n p) c -> p n c", p=128))
        sc_t = sb.tile([128, NT, 64], F32)
        nc.sync.dma_start(out=sc_t[:], in_=d_sc.rearrange("(n p) c -> p n c", p=128))
        wo_h = [sb.tile([64, D], F32R, tag=f"wo{h}", name=f"wo{h}") for h in range(2)]
        nc.sync.dma_start(out=wo_h[0][:], in_=d_wo0[:])
        nc.sync.dma_start(out=wo_h[1][:], in_=d_wo1[:])
        idn = sb.tile([128, 128], F32R)
        nc.sync.dma_start(out=idn[:], in_=d_idn[:])
        tri = sb.tile([128, 128], F32)
        nc.sync.dma_start(out=tri[:], in_=d_tri[:])
        esk = sb.tile([1, 130], F32R)
        nc.sync.dma_start(out=esk[:], in_=d_esk[:])
        on1 = sb.tile([1, 512], F32R)
        nc.sync.dma_start(out=on1[:], in_=d_on1[:])
        onp = sb.tile([65, 64], F32R)
        nc.sync.dma_start(out=onp[:], in_=d_onp[:])
        on2 = sb.tile([128, 2], F32)
        nc.sync.dma_start(out=on2[:], in_=d_on2[:])

        epst = sb.tile([128, 1], F32)
        nc.gpsimd.memset(epst[:], RMS_EPS)
        ln8t = sb.tile([128, 1], F32)
        nc.gpsimd.memset(ln8t[:], LN8)

        stats = sb.tile([128, 64], F32)
        lnst = sb.tile([128, 64], F32)
        rbuf = sb.tile([128, 64], F32)
        qT = sb.tile([128, T], F32R)
        kT = sb.tile([128, T], F32R)
        vtiles = [sb.tile([128, 130], F32R, tag=f"v{i}", name=f"v{i}") for i in range(NT)]
        qkr = [sb.tile([128, 256], F32R, tag=f"qkr{i}", name=f"qkr{i}") for i in range(NT)]
        qkro = [sb.tile([128, 256], F32, tag=f"qkro{i}", name=f"qkro{i}") for i in range(NT)]
        lsb = sb.tile([65, 8, 512], F32R)
        yts = [sb.tile([64, T], F32R, tag=f"yts{h}", name=f"yts{h}") for h in range(2)]

        # ---------------- P1: fused QKV + stats + rope + v-blend ----------
        with tc.tile_pool(name="ps_qkv", bufs=2, space="PSUM") as ps_qkv:
            for ti in range(NT):
                xt = sb_x.tile([128, 8, 128], F32R, tag="xt")
                nc.sync.dma_start(
                    out=xt[:],
                    in_=d_xT[:, 128 * ti : 128 * (ti + 1)].rearrange(
                        "(n p) c -> p n c", p=128
                    ),
                )
                psq = ps_qkv.tile([128, 384], F32)
                for i in range(8):
                    nc.tensor.matmul(
                        psq[:],
                        xt[:, i, :],
                        wqkv[:, i, :],
                        start=(i == 0),
                        stop=(i == 7),
                    )
                # stats: Square (ACT) + segmented reduce (DVE)
                sqt = sb_w1.tile([128, 256], F32, tag="sqt")
                nc.scalar.activation(sqt[:], psq[:, 0:256], AF.Square)
                nc.vector.tensor_reduce(
                    stats[:, 4 * ti : 4 * ti + 4],
                    sqt[:].rearrange("p (s c) -> p s c", s=4),
                    axis=AX.X,
                    op=ALU.add,
                )
                # v-blend fused into PSUM evac + ones cols
                vt = vtiles[ti]
                nc.vector.tensor_tensor(
                    out=vt[:].rearrange("p (s c) -> p s c", s=2)[:, :, 0:64],
                    in0=psq[:, 256:384].rearrange("p (s c) -> p s c", s=2),
                    in1=vi_t[:, ti, :].rearrange("p (s c) -> p s c", s=2),
                    op=ALU.add,
                )
                nc.vector.tensor_copy(
                    vt[:].rearrange("p (s c) -> p s c", s=2)[:, :, 64:65],
                    on2[:].rearrange("p (s c) -> p s c", s=2),
                )
                # rope: evac (ACT) -> swap (GPSIMD) -> mult1 (DVE) -> mult2
                # (GPSIMD) -> add (DVE)
                qkt = sb_w1.tile([128, 256], F32, tag="qkt")
                nc.scalar.copy(qkt[:], psq[:, 0:256])
                qksw = sb_w1.tile([128, 256], F32, tag="qksw")
                qk4 = qkt[:].rearrange("p (s h c) -> p s h c", s=4, h=2)
                sw4 = qksw[:].rearrange("p (s h c) -> p s h c", s=4, h=2)
                nc.gpsimd.tensor_copy(sw4[:, :, 0, :], qk4[:, :, 1, :])
                nc.gpsimd.tensor_copy(sw4[:, :, 1, :], qk4[:, :, 0, :])
                tcos = sb_w2.tile([128, 256], F32, tag="tcos")
                nc.vector.tensor_tensor(
                    out=tcos[:].rearrange("p (s c) -> p s c", s=4),
                    in0=qkt[:].rearrange("p (s c) -> p s c", s=4),
                    in1=cc_t[:, ti, :].unsqueeze(1).broadcast_to((128, 4, 64)),
                    op=ALU.mult,
                )
                tsin = sb_w2.tile([128, 256], F32, tag="tsin")
                nc.gpsimd.tensor_tensor(
                    out=tsin[:].rearrange("p (s c) -> p s c", s=4),
                    in0=qksw[:].rearrange("p (s c) -> p s c", s=4),
                    in1=sc_t[:, ti, :].unsqueeze(1).broadcast_to((128, 4, 64)),
                    op=ALU.mult,
                )
                nc.vector.tensor_tensor(
                    out=qkro[ti][:], in0=tcos[:], in1=tsin[:], op=ALU.add
                )
                # batched rsqrt per group of 4 tiles, then normalization apply
                if ti % 4 == 3:
                    g = 16 * (ti // 4)
                    nc.scalar.activation(
                        lnst[:, g : g + 16],
                        stats[:, g : g + 16],
                        AF.Ln,
                        scale=1.0 / 64.0,
                        bias=epst[:],
                    )
                    lv = lnst[:, g : g + 16].rearrange("p (t c) -> p t c", c=4)
                    rv = rbuf[:, g : g + 16].rearrange("p (t c) -> p t c", c=4)
                    nc.scalar.activation(rv[:, :, 0:2], lv[:, :, 0:2], AF.Exp, scale=-0.5)
                    nc.scalar.activation(
                        rv[:, :, 2:4], lv[:, :, 2:4], AF.Exp, scale=-0.5, bias=ln8t[:]
                    )
                    for tj in range(ti - 3, ti + 1):
                        for s in range(4):  # qA qB kA kB
                            nc.vector.tensor_scalar_mul(
                                qkr[tj][:, 64 * s : 64 * s + 64],
                                qkro[tj][:, 64 * s : 64 * s + 64],
                                rbuf[:, 4 * tj + s : 4 * tj + s + 1],
                            )
        # ---------------- P2: transposes -> qT, kT ------------------------
        with tc.tile_pool(name="ps_tr", bufs=2, space="PSUM") as ps_tr:
            for ti in range(NT):
                for which, dst in ((0, qT), (1, kT)):
                    ptr = ps_tr.tile([128, 128], F32R, tag="tr")
                    nc.tensor.transpose(
                        ptr[:], qkr[ti][:, 128 * which : 128 * which + 128], idn[:]
                    )
                    nc.vector.tensor_copy(
                        dst[:, 128 * ti : 128 * (ti + 1)], ptr[:]
                    )

        # ---------------- P3 + P4: attention, scale, out-proj ------------
        with (
            tc.tile_pool(name="ps_st", bufs=2, space="PSUM") as ps_st,
            tc.tile_pool(name="ps_yt", bufs=2, space="PSUM") as ps_yt,
            tc.tile_pool(name="ps_mo", bufs=1, space="PSUM") as ps_mo,
        ):
            for ci in range(4):
                yt_h = []
                for h in range(2):
                    kj_max = 4 * ci + 4
                    yt = ps_yt.tile([128, 512], F32, tag="yt")
                    yt_h.append(yt)
                    for dd in range(kj_max // 2):
                        st = ps_st.tile([128, 1024], F32, tag="st")
                        et = sb_e.tile([128, 1024], F32R, tag="et")
                        qs = {}
                        for j2 in range(2):
                            kj = 2 * dd + j2
                            qs[j2] = 128 * (kj - 4 * ci) if kj >= 4 * ci else 0
                            nc.tensor.matmul(
                                st[:, 512 * j2 + qs[j2] : 512 * (j2 + 1)],
                                kT[64 * h : 64 * h + 64, 128 * kj : 128 * (kj + 1)],
                                qT[64 * h : 64 * h + 64, 512 * ci + qs[j2] : 512 * (ci + 1)],
                                start=True,
                                stop=True,
                            )
                        nc.scalar.activation(et[:], st[:], AF.Exp)
                        for j2 in range(2):
                            kj = 2 * dd + j2
                            if kj >= 4 * ci:  # diagonal: tri-mask the block
                                blk = et[:, 512 * j2 + qs[j2] : 512 * j2 + qs[j2] + 128]
                                nc.vector.tensor_tensor(
                                    out=blk, in0=blk.bitcast(F32), in1=tri[:], op=ALU.mult
                                )
                        for j2 in range(2):
                            kj = 2 * dd + j2
                            nc.tensor.matmul(
                                yt[0:65, qs[j2] : 512],
                                vtiles[kj][:, 65 * h : 65 * h + 65],
                                et[:, 512 * j2 + qs[j2] : 512 * (j2 + 1)],
                                start=(kj == 0),
                                stop=False,
                            )
                    # + e^sink into the L row (row 64)
                    nc.tensor.matmul(
                        yt[0:65, 0:512],
                        esk[:, 65 * h : 65 * h + 65],
                        on1[:],
                        start=False,
                        stop=True,
                    )
                # P4 for this ci
                mb = ps_mo.tile([64, 1024], F32, tag="mo")
                mbs = sb_w2.tile([64, 1024], F32, tag="mbs")
                for h in range(2):
                    nc.vector.tensor_copy(
                        lsb[64:65, 4 * h + ci, :], yt_h[h][64:65, 0:512]
                    )
                    nc.tensor.matmul(
                        mb[0:64, 512 * h : 512 * h + 512],
                        onp[64:65, 0:64],
                        lsb[64:65, 4 * h + ci, :],
                        start=True,
                        stop=True,
                    )
                    nc.vector.reciprocal(
                        mbs[0:64, 512 * h : 512 * h + 512],
                        mb[0:64, 512 * h : 512 * h + 512],
                    )
                    nc.vector.tensor_tensor(
                        out=yts[h][0:64, 512 * ci : 512 * (ci + 1)],
                        in0=yt_h[h][0:64, 0:512],
                        in1=mbs[0:64, 512 * h : 512 * h + 512],
                        op=ALU.mult,
                    )
                for tt in range(4 * ci, 4 * ci + 4):
                    pso = ps_mo.tile([128, 1024], F32, tag="mo")
                    for jc in range(2):
                        for h in range(2):
                            nc.tensor.matmul(
                                pso[:, 512 * jc : 512 * (jc + 1)],
                                yts[h][0:64, 128 * tt : 128 * (tt + 1)],
                                wo_h[h][0:64, 512 * jc : 512 * (jc + 1)],
                                start=(h == 0),
                                stop=(h == 1),
                            )
                    outsb = sb_o.tile([128, 1024], F32, tag="outsb")
                    if tt % 2 == 0:
                        nc.vector.tensor_copy(outsb[:], pso[:])
                    else:
                        nc.scalar.copy(outsb[:], pso[:])
                    nc.sync.dma_start(
                        out=d_out[128 * tt : 128 * (tt + 1), :], in_=outsb[:]
                    )

    nc.compile()
    return nc


_NC = None


def _rope_tables():
    inv = (1.0 / 10000.0) ** (np.arange(0, HD, 2, dtype=np.float64) / HD)
    t = np.arange(T, dtype=np.float64)
    f = np.outer(t, inv)  # (T, 32)
    cc = np.concatenate([np.cos(f), np.cos(f)], axis=1).astype(np.float32)
    sc = np.concatenate([np.sin(f), -np.sin(f)], axis=1).astype(np.float32)
    return cc, sc


def kernel(x, vi, Wq, Wk, Wv, Wo, lamb, sink_weights):
    global _NC
    x = np.asarray(x, dtype=np.float32)
    vi = np.asarray(vi, dtype=np.float32)
    Wq = np.asarray(Wq, dtype=np.float32)
    Wk = np.asarray(Wk, dtype=np.float32)
    Wv = np.asarray(Wv, dtype=np.float32)
    Wo = np.asarray(Wo, dtype=np.float32)
    lam = float(np.asarray(lamb).reshape(-1)[0])
    sink = np.asarray(sink_weights, dtype=np.float32).reshape(-1)

    if _NC is None:
        _NC = _build_program()

    xT = np.ascontiguousarray(x[0].T)  # (D, T)
    cc, sc = _rope_tables()
    tri = (np.arange(128)[None, :] >= np.arange(128)[:, None]).astype(np.float32)
    idn = np.eye(128, dtype=np.float32)
    on1 = np.ones((1, 512), np.float32)
    onp = np.zeros((65, 64), np.float32)
    onp[64, :] = 1.0
    on2 = np.ones((128, 2), np.float32)

    in_maps = []
    for c in range(8):
        lo = 128 * c
        wqkv = np.ascontiguousarray(
            np.concatenate(
                [
                    Wq[lo : lo + 128].T,
                    Wk[lo : lo + 128].T,
                    (1.0 - lam) * Wv[lo : lo + 128].T,
                ],
                axis=1,
            )
        )  # (D, 384)
        esk = np.zeros((1, 130), np.float32)
        esk[0, 64] = np.exp(sink[2 * c])
        esk[0, 129] = np.exp(sink[2 * c + 1])
        in_maps.append(
            {
                "xT": xT,
                "wqkv": wqkv,
                "vis": np.ascontiguousarray(lam * vi[0][:, lo : lo + 128]),
                "cc": cc,
                "sc": sc,
                "wo0": np.ascontiguousarray(Wo[:, lo : lo + 64].T),
                "wo1": np.ascontiguousarray(Wo[:, lo + 64 : lo + 128].T),
                "idn": idn,
                "tri": tri,
                "esk": esk,
                "on1": on1,
                "onp": onp,
                "on2": on2,
            }
        )

    global _trace_in_maps
    _trace_in_maps = in_maps
    res = run_bass_kernel_spmd(_NC, in_maps, list(range(8)))
    out = np.zeros((T, D), np.float64)
    for c in range(8):
        out += res.results[c]["out"]
    return out.astype(np.float32).reshape(1, T, D)


# revision 7
# speedup vs baseline: 1.0561x; 1.0561x over previous
"""Trainium2 Bass kernel for nn_CausalSelfAttention (B=1, T=2048, D=1024, H=16).

Sharding: 2 heads per core across 8 cores (tensor parallel). Wq/Wk/Wv
column-sharded by head, attention fully local, Wo row-sharded; host sums the
8 partial outputs (the all-reduce of the unshard step).

Per-core pipeline (all matmuls fp32r):
  P1  fused QKV: psum[t,384] = sum_i xT_blk.T @ [WqT|WkT|(1-l)WvT]; RMS stats
      (Square+segmented reduce -> Ln -> Exp = rsqrt, 0.125 folded into k's
      scale), RoPE via concat-table trick (y = qk*ccat + swap(qk)*scat),
      v-blend (+lam*vi) fused into PSUM evacuation, [vA|1|vB|1] tiles.
  P2  PE-transpose roped q,k -> qT,kT (d-major, f32r).
  P3  per (ci, head): ST[tk,tq] = kT_slice.T @ qT_chunk into 2-bank PSUM duos,
      one Exp per duo, tri-mask on diagonal blocks, matmul2 YT[d|L,tq] with
      lhsT=[v|1] and rhs=E (no P transposes), e^sink accumulated via K=1 matmul
      so scale = sigmoid(lse-sink)/L = 1/(L + e^sink).
  P4  broadcast 1/(L+e^sink) across partitions via K=1 matmul + reciprocal,
      scale YT -> yTs_h, out-proj per head (K=64) accumulating, evacuate, DMA.
"""

import sys

if "/opt/trn_rl_repo" not in sys.path:
    sys.path.insert(0, "/opt/trn_rl_repo")

import numpy as np
from contextlib import ExitStack

from concourse import bacc, tile
from concourse import mybir
from concourse.bass_utils import run_bass_kernel_spmd

F32 = mybir.dt.float32
F32R = mybir.dt.float32r
I32 = mybir.dt.int32
AF = mybir.ActivationFunctionType
ALU = mybir.AluOpType
AX = mybir.AxisListType

T = 2048
D = 1024
HD = 64
NT = T // 128  # 16 t-tiles
RMS_EPS = float(np.finfo(np.float32).eps)
LN8 = float(np.log(0.125))


def _build_program():
    nc = bacc.Bacc("TRN2", target_bir_lowering=False, debug=False, num_devices=8)

    d_xT = nc.dram_tensor("xT", [D, T], F32R, kind="ExternalInput").ap()
    d_wqkv = nc.dram_tensor("wqkv", [D, 384], F32R, kind="ExternalInput").ap()
    d_vis = nc.dram_tensor("vis", [T, 128], F32, kind="ExternalInput").ap()
    d_cc = nc.dram_tensor("cc", [T, 64], F32, kind="ExternalInput").ap()
    d_sc = nc.dram_tensor("sc", [T, 64], F32, kind="ExternalInput").ap()
    d_wo0 = nc.dram_tensor("wo0", [64, D], F32R, kind="ExternalInput").ap()
    d_wo1 = nc.dram_tensor("wo1", [64, D], F32R, kind="ExternalInput").ap()
    d_idn = nc.dram_tensor("idn", [128, 128], F32R, kind="ExternalInput").ap()
    d_tri = nc.dram_tensor("tri", [128, 128], F32, kind="ExternalInput").ap()
    d_esk = nc.dram_tensor("esk", [1, 130], F32R, kind="ExternalInput").ap()
    d_on1 = nc.dram_tensor("on1", [1, 512], F32R, kind="ExternalInput").ap()
    d_onp = nc.dram_tensor("onp", [65, 64], F32R, kind="ExternalInput").ap()
    d_on2 = nc.dram_tensor("on2", [128, 2], F32, kind="ExternalInput").ap()
    d_out = nc.dram_tensor("out", [T, D], F32, kind="ExternalOutput").ap()

    with tile.TileContext(nc) as tc, ExitStack() as ctx:
        sb = ctx.enter_context(tc.tile_pool(name="sb", bufs=1))
        sb_x = ctx.enter_context(tc.tile_pool(name="sb_x", bufs=4))
        sb_w1 = ctx.enter_context(tc.tile_pool(name="sb_w1", bufs=3))
        sb_w2 = ctx.enter_context(tc.tile_pool(name="sb_w2", bufs=3))
        sb_e = ctx.enter_context(tc.tile_pool(name="sb_e", bufs=3))
        sb_o = ctx.enter_context(tc.tile_pool(name="sb_o", bufs=3))

        wqkv = sb.tile([128, 8, 384], F32R)
        nc.sync.dma_start(out=wqkv[:], in_=d_wqkv.rearrange("(n p) c -> p n c", p=128))
        vi_t = sb.tile([128, NT, 128], F32)
        nc.sync.dma_start(out=vi_t[:], in_=d_vis.rearrange("(n p) c -> p n c", p=128))
        cc_t = sb.tile([128, NT, 64], F32)
        nc.sync.dma_start(out=cc_t[:], in_=d_cc.rearrange("(n p) c -> p n c", p=128))
        sc_t = sb.tile([128, NT, 64], F32)
        nc.sync.dma_start(out=sc_t[:], in_=d_sc.rearrange("(n p) c -> p n c", p=128))
        wo_h = [sb.tile([64, D], F32R, tag=f"wo{h}", name=f"wo{h}") for h in range(2)]
        nc.sync.dma_start(out=wo_h[0][:], in_=d_wo0[:])
        nc.sync.dma_start(out=wo_h[1][:], in_=d_wo1[:])
        idn = sb.tile([128, 128], F32R)
        nc.sync.dma_start(out=idn[:], in_=d_idn[:])
        tri = sb.tile([128, 128], F32)
        nc.sync.dma_start(out=tri[:], in_=d_tri[:])
        esk = sb.tile([1, 130], F32R)
        nc.sync.dma_start(out=esk[:], in_=d_esk[:])
        on1 = sb.tile([1, 512], F32R)
        nc.sync.dma_start(out=on1[:], in_=d_on1[:])
        onp = sb.tile([65, 64], F32R)
        nc.sync.dma_start(out=onp[:], in_=d_onp[:])
        on2 = sb.tile([128, 2], F32)
        nc.sync.dma_start(out=on2[:], in_=d_on2[:])

        stats = sb.tile([128, 64], F32)
        rbuf = sb.tile([128, 64], F32)
        qT = sb.tile([128, T], F32R)
        kT = sb.tile([128, T], F32R)
        vtiles = [sb.tile([128, 130], F32R, tag=f"v{i}", name=f"v{i}") for i in range(NT)]
        qkr = [sb.tile([128, 256], F32R, tag=f"qkr{i}", name=f"qkr{i}") for i in range(NT)]
        qkro = [sb.tile([128, 256], F32, tag=f"qkro{i}", name=f"qkro{i}") for i in range(NT)]
        lsb = sb.tile([65, 8, 512], F32R)
        yts = [sb.tile([64, T], F32R, tag=f"yts{h}", name=f"yts{h}") for h in range(2)]

        # ---------------- P1: fused QKV + stats + rope + v-blend ----------
        with tc.tile_pool(name="ps_qkv", bufs=2, space="PSUM") as ps_qkv:
            for ti in range(NT):
                xt = sb_x.tile([128, 8, 128], F32R, tag="xt")
                nc.sync.dma_start(
                    out=xt[:],
                    in_=d_xT[:, 128 * ti : 128 * (ti + 1)].rearrange(
                        "(n p) c -> p n c", p=128
                    ),
                )
                psq = ps_qkv.tile([128, 384], F32)
                for i in range(8):
                    nc.tensor.matmul(
                        psq[:],
                        xt[:, i, :],
                        wqkv[:, i, :],
                        start=(i == 0),
                        stop=(i == 7),
                    )
                # stats: Square (ACT) + segmented reduce (DVE)
                sqt = sb_w1.tile([128, 256], F32, tag="sqt")
                nc.scalar.activation(sqt[:], psq[:, 0:256], AF.Square)
                nc.vector.tensor_reduce(
                    stats[:, 4 * ti : 4 * ti + 4],
                    sqt[:].rearrange("p (s c) -> p s c", s=4),
                    axis=AX.X,
                    op=ALU.add,
                )
                # v-blend fused into PSUM evac + ones cols
                vt = vtiles[ti]
                nc.vector.tensor_tensor(
                    out=vt[:].rearrange("p (s c) -> p s c", s=2)[:, :, 0:64],
                    in0=psq[:, 256:384].rearrange("p (s c) -> p s c", s=2),
                    in1=vi_t[:, ti, :].rearrange("p (s c) -> p s c", s=2),
                    op=ALU.add,
                )
                nc.vector.tensor_copy(
                    vt[:].rearrange("p (s c) -> p s c", s=2)[:, :, 64:65],
                    on2[:].rearrange("p (s c) -> p s c", s=2),
                )
                # rope: evac (ACT) -> swap (GPSIMD) -> mult1 (DVE) -> mult2
                # (GPSIMD) -> add (DVE)
                qkt = sb_w1.tile([128, 256], F32, tag="qkt")
                nc.scalar.copy(qkt[:], psq[:, 0:256])
                qksw = sb_w1.tile([128, 256], F32, tag="qksw")
                qk4 = qkt[:].rearrange("p (s h c) -> p s h c", s=4, h=2)
                sw4 = qksw[:].rearrange("p (s h c) -> p s h c", s=4, h=2)
                nc.gpsimd.tensor_copy(sw4[:, :, 0, :], qk4[:, :, 1, :])
                nc.gpsimd.tensor_copy(sw4[:, :, 1, :], qk4[:, :, 0, :])
                tcos = sb_w2.tile([128, 256], F32, tag="tcos")
                nc.vector.tensor_tensor(
                    out=tcos[:].rearrange("p (s c) -> p s c", s=4),
                    in0=qkt[:].rearrange("p (s c) -> p s c", s=4),
                    in1=cc_t[:, ti, :].unsqueeze(1).broadcast_to((128, 4, 64)),
                    op=ALU.mult,
                )
                tsin = sb_w2.tile([128, 256], F32, tag="tsin")
                nc.gpsimd.tensor_tensor(
                    out=tsin[:].rearrange("p (s c) -> p s c", s=4),
                    in0=qksw[:].rearrange("p (s c) -> p s c", s=4),
                    in1=sc_t[:, ti, :].unsqueeze(1).broadcast_to((128, 4, 64)),
                    op=ALU.mult,
                )
                nc.vector.tensor_tensor(
                    out=qkro[ti][:], in0=tcos[:], in1=tsin[:], op=ALU.add
                )
                # batched rsqrt per group of 4 tiles (DVE bit-trick +
                # 2 Newton iters; keeps ACT on the exp table only), then apply
                if ti % 4 == 3:
                    g = 16 * (ti // 4)
                    rs = rbuf[:, g : g + 16]
                    zt = sb_w2.tile([128, 16], F32, tag="zt", name=f"zt{ti}")
                    nt1 = sb_w2.tile([128, 16], F32, tag="nt1", name=f"nt1{ti}")
                    nc.vector.tensor_scalar(
                        out=zt[:], in0=stats[:, g : g + 16], scalar1=1.0 / 64.0,
                        scalar2=RMS_EPS, op0=ALU.mult, op1=ALU.add,
                    )
                    nc.vector.tensor_scalar(
                        out=nt1[:].bitcast(I32), in0=zt[:].bitcast(I32), scalar1=1,
                        scalar2=0xFFFFFFFF, op0=ALU.logical_shift_right,
                        op1=ALU.bitwise_xor,
                    )
                    nc.vector.tensor_scalar(
                        out=rs.bitcast(I32), in0=nt1[:].bitcast(I32),
                        scalar1=0x5F3759E0, scalar2=None, op0=ALU.add,
                    )
                    for _ in range(2):
                        nc.vector.tensor_tensor(out=nt1[:], in0=rs, in1=rs, op=ALU.mult)
                        nc.vector.tensor_tensor(out=nt1[:], in0=nt1[:], in1=zt[:], op=ALU.mult)
                        nc.vector.tensor_scalar(
                            out=nt1[:], in0=nt1[:], scalar1=-0.5, scalar2=1.5,
                            op0=ALU.mult, op1=ALU.add,
                        )
                        nc.vector.tensor_tensor(out=rs, in0=rs, in1=nt1[:], op=ALU.mult)
                    for tj in range(ti - 3, ti + 1):
                        for s in range(4):  # qA qB kA kB (k gets 0.125 folded)
                            if s < 2:
                                nc.vector.tensor_scalar_mul(
                                    qkr[tj][:, 64 * s : 64 * s + 64],
                                    qkro[tj][:, 64 * s : 64 * s + 64],
                                    rbuf[:, 4 * tj + s : 4 * tj + s + 1],
                                )
                            else:
                                nc.vector.tensor_scalar(
                                    out=qkr[tj][:, 64 * s : 64 * s + 64],
                                    in0=qkro[tj][:, 64 * s : 64 * s + 64],
                                    scalar1=rbuf[:, 4 * tj + s : 4 * tj + s + 1],
                                    scalar2=0.125, op0=ALU.mult, op1=ALU.mult,
                                )
        # ---------------- P2: transposes -> qT, kT ------------------------
        with tc.tile_pool(name="ps_tr", bufs=2, space="PSUM") as ps_tr:
            for ti in range(NT):
                for which, dst in ((0, qT), (1, kT)):
                    ptr = ps_tr.tile([128, 128], F32R, tag="tr")
                    nc.tensor.transpose(
                        ptr[:], qkr[ti][:, 128 * which : 128 * which + 128], idn[:]
                    )
                    nc.vector.tensor_copy(
                        dst[:, 128 * ti : 128 * (ti + 1)], ptr[:]
                    )

        # ---------------- P3 + P4: attention, scale, out-proj ------------
        with (
            tc.tile_pool(name="ps_st", bufs=2, space="PSUM") as ps_st,
            tc.tile_pool(name="ps_yt", bufs=2, space="PSUM") as ps_yt,
            tc.tile_pool(name="ps_mo", bufs=2, space="PSUM") as ps_mo,
        ):
            for ci in range(4):
                yt_h = []
                for h in range(2):
                    kj_max = 4 * ci + 4
                    yt = ps_yt.tile([128, 512], F32, tag="yt")
                    yt_h.append(yt)
                    for dd in range(kj_max // 2):
                        st = ps_st.tile([128, 1024], F32, tag="st")
                        et = sb_e.tile([128, 1024], F32R, tag="et")
                        qs = {}
                        for j2 in range(2):
                            kj = 2 * dd + j2
                            qs[j2] = 128 * (kj - 4 * ci) if kj >= 4 * ci else 0
                            nc.tensor.matmul(
                                st[:, 512 * j2 + qs[j2] : 512 * (j2 + 1)],
                                kT[64 * h : 64 * h + 64, 128 * kj : 128 * (kj + 1)],
                                qT[64 * h : 64 * h + 64, 512 * ci + qs[j2] : 512 * (ci + 1)],
                                start=True,
                                stop=True,
                            )
                        nc.scalar.activation(et[:], st[:], AF.Exp)
                        for j2 in range(2):
                            kj = 2 * dd + j2
                            if kj >= 4 * ci:  # diagonal: tri-mask the block
                                blk = et[:, 512 * j2 + qs[j2] : 512 * j2 + qs[j2] + 128]
                                nc.gpsimd.tensor_tensor(
                                    out=blk, in0=blk.bitcast(F32), in1=tri[:], op=ALU.mult
                                )
                        for j2 in range(2):
                            kj = 2 * dd + j2
                            nc.tensor.matmul(
                                yt[0:65, qs[j2] : 512],
                                vtiles[kj][:, 65 * h : 65 * h + 65],
                                et[:, 512 * j2 + qs[j2] : 512 * (j2 + 1)],
                                start=(kj == 0),
                                stop=False,
                            )
                    # + e^sink into the L row (row 64)
                    nc.tensor.matmul(
                        yt[0:65, 0:512],
                        esk[:, 65 * h : 65 * h + 65],
                        on1[:],
                        start=False,
                        stop=True,
                    )
                # P4 for this ci
                mb = ps_st.tile([128, 1024], F32, tag="st", name=f"mb{ci}")
                mbs = sb_w2.tile([64, 1024], F32, tag="mbs", name=f"mbs{ci}")
                for h in range(2):
                    nc.vector.tensor_copy(
                        lsb[64:65, 4 * h + ci, :], yt_h[h][64:65, 0:512]
                    )
                    nc.tensor.matmul(
                        mb[0:64, 512 * h : 512 * h + 512],
                        onp[64:65, 0:64],
                        lsb[64:65, 4 * h + ci, :],
                        start=True,
                        stop=True,
                    )
                    nc.vector.reciprocal_approx_fast(
                        out=mbs[0:64, 512 * h : 512 * h + 512],
                        in_=mb[0:64, 512 * h : 512 * h + 512],
                    )
                    nc.vector.tensor_tensor(
                        out=yts[h][0:64, 512 * ci : 512 * (ci + 1)],
                        in0=yt_h[h][0:64, 0:512],
                        in1=mbs[0:64, 512 * h : 512 * h + 512],
                        op=ALU.mult,
                    )
                for tt in range(4 * ci, 4 * ci + 4):
                    for jc in range(2):
                        pso = ps_mo.tile(
                            [128, 512], F32, tag="mo", name=f"pso{tt}_{jc}"
                        )
                        for h in range(2):
                            nc.tensor.matmul(
                                pso[:],
                                yts[h][0:64, 128 * tt : 128 * (tt + 1)],
                                wo_h[h][0:64, 512 * jc : 512 * (jc + 1)],
                                start=(h == 0),
                                stop=(h == 1),
                            )
                        outsb = sb_o.tile(
                            [128, 512], F32, tag="outsb", name=f"osb{tt}_{jc}"
                        )
                        if jc == 0:
                            nc.vector.tensor_copy(outsb[:], pso[:])
                        else:
                            nc.scalar.copy(outsb[:], pso[:])
                        nc.sync.dma_start(
                            out=d_out[
                                128 * tt : 128 * (tt + 1),
                                512 * jc : 512 * (jc + 1),
                            ],
                            in_=outsb[:],
                        )

    nc.compile()
    return nc


_NC = None


def _rope_tables():
    inv = (1.0 / 10000.0) ** (np.arange(0, HD, 2, dtype=np.float64) / HD)
    t = np.arange(T, dtype=np.float64)
    f = np.outer(t, inv)  # (T, 32)
    cc = np.concatenate([np.cos(f), np.cos(f)], axis=1).astype(np.float32)
    sc = np.concatenate([np.sin(f), -np.sin(f)], axis=1).astype(np.float32)
    return cc, sc


def kernel(x, vi, Wq, Wk, Wv, Wo, lamb, sink_weights):
    global _NC
    x = np.asarray(x, dtype=np.float32)
    vi = np.asarray(vi, dtype=np.float32)
    Wq = np.asarray(Wq, dtype=np.float32)
    Wk = np.asarray(Wk, dtype=np.float32)
    Wv = np.asarray(Wv, dtype=np.float32)
    Wo = np.asarray(Wo, dtype=np.float32)
    lam = float(np.asarray(lamb).reshape(-1)[0])
    sink = np.asarray(sink_weights, dtype=np.float32).reshape(-1)

    if _NC is None:
        _NC = _build_program()

    xT = np.ascontiguousarray(x[0].T)  # (D, T)
    cc, sc = _rope_tables()
    tri = (np.arange(128)[None, :] >= np.arange(128)[:, None]).astype(np.float32)
    idn = np.eye(128, dtype=np.float32)
    on1 = np.ones((1, 512), np.float32)
    onp = np.zeros((65, 64), np.float32)
    onp[64, :] = 1.0
    on2 = np.ones((128, 2), np.float32)

    in_maps = []
    for c in range(8):
        lo = 128 * c
        wqkv = np.ascontiguousarray(
            np.concatenate(
                [
                    Wq[lo : lo + 128].T,
                    Wk[lo : lo + 128].T,
                    (1.0 - lam) * Wv[lo : lo + 128].T,
                ],
                axis=1,
            )
        )  # (D, 384)
        esk = np.zeros((1, 130), np.float32)
        esk[0, 64] = np.exp(sink[2 * c])
        esk[0, 129] = np.exp(sink[2 * c + 1])
        in_maps.append(
            {
                "xT": xT,
                "wqkv": wqkv,
                "vis": np.ascontiguousarray(lam * vi[0][:, lo : lo + 128]),
                "cc": cc,
                "sc": sc,
                "wo0": np.ascontiguousarray(Wo[:, lo : lo + 64].T),
                "wo1": np.ascontiguousarray(Wo[:, lo + 64 : lo + 128].T),
                "idn": idn,
                "tri": tri,
                "esk": esk,
                "on1": on1,
                "onp": onp,
                "on2": on2,
            }
        )

    global _trace_in_maps
    _trace_in_maps = in_maps
    res = run_bass_kernel_spmd(_NC, in_maps, list(range(8)))
    out = np.zeros((T, D), np.float64)
    for c in range(8):
        out += res.results[c]["out"]
    return out.astype(np.float32).reshape(1, T, D)


# revision 8
# speedup vs baseline: 1.1197x; 1.0602x over previous
"""Trainium2 Bass kernel for nn_CausalSelfAttention (B=1, T=2048, D=1024, H=16).

Sharding: 2 heads per core across 8 cores (tensor parallel). Wq/Wk/Wv
column-sharded by head, attention fully local, Wo row-sharded; host sums the
8 partial outputs (the all-reduce of the unshard step).

Per-core pipeline (all matmuls fp32r):
  P1  fused QKV: psum[t,384] = sum_i xT_blk.T @ [WqT|WkT|(1-l)WvT]; RMS stats
      (Square+segmented reduce -> Ln -> Exp = rsqrt, 0.125 folded into k's
      scale), RoPE via concat-table trick (y = qk*ccat + swap(qk)*scat),
      v-blend (+lam*vi) fused into PSUM evacuation, [vA|1|vB|1] tiles.
  P2  PE-transpose roped q,k -> qT,kT (d-major, f32r).
  P3  per (ci, head): ST[tk,tq] = kT_slice.T @ qT_chunk into 2-bank PSUM duos,
      one Exp per duo, tri-mask on diagonal blocks, matmul2 YT[d|L,tq] with
      lhsT=[v|1] and rhs=E (no P transposes), e^sink accumulated via K=1 matmul
      so scale = sigmoid(lse-sink)/L = 1/(L + e^sink).
  P4  broadcast 1/(L+e^sink) across partitions via K=1 matmul + reciprocal,
      scale YT -> yTs_h, out-proj per head (K=64) accumulating, evacuate, DMA.
"""

import sys

if "/opt/trn_rl_repo" not in sys.path:
    sys.path.insert(0, "/opt/trn_rl_repo")

import numpy as np
from contextlib import ExitStack

from concourse import bacc, tile
from concourse import mybir
from concourse.bass_utils import run_bass_kernel_spmd

F32 = mybir.dt.float32
F32R = mybir.dt.float32r
I32 = mybir.dt.int32
AF = mybir.ActivationFunctionType
ALU = mybir.AluOpType
AX = mybir.AxisListType

T = 2048
D = 1024
HD = 64
NT = T // 128  # 16 t-tiles
RMS_EPS = float(np.finfo(np.float32).eps)
LN8 = float(np.log(0.125))


def _build_program():
    nc = bacc.Bacc("TRN2", target_bir_lowering=False, debug=False, num_devices=8)

    d_xT = nc.dram_tensor("xT", [D, T], F32R, kind="ExternalInput").ap()
    d_wqkv = nc.dram_tensor("wqkv", [D, 384], F32R, kind="ExternalInput").ap()
    d_vis = nc.dram_tensor("vis", [T, 128], F32, kind="ExternalInput").ap()
    d_cc = nc.dram_tensor("cc", [T, 64], F32, kind="ExternalInput").ap()
    d_sc = nc.dram_tensor("sc", [T, 64], F32, kind="ExternalInput").ap()
    d_wo0 = nc.dram_tensor("wo0", [64, D], F32R, kind="ExternalInput").ap()
    d_wo1 = nc.dram_tensor("wo1", [64, D], F32R, kind="ExternalInput").ap()
    d_idn = nc.dram_tensor("idn", [128, 128], F32R, kind="ExternalInput").ap()
    d_tri = nc.dram_tensor("tri", [128, 128], F32, kind="ExternalInput").ap()
    d_esk = nc.dram_tensor("esk", [1, 130], F32R, kind="ExternalInput").ap()
    d_on1 = nc.dram_tensor("on1", [1, 512], F32R, kind="ExternalInput").ap()
    d_onp = nc.dram_tensor("onp", [65, 64], F32R, kind="ExternalInput").ap()
    d_on2 = nc.dram_tensor("on2", [128, 2], F32, kind="ExternalInput").ap()
    d_out = nc.dram_tensor("out", [T, D], F32, kind="ExternalOutput").ap()

    with tile.TileContext(nc) as tc, ExitStack() as ctx:
        sb = ctx.enter_context(tc.tile_pool(name="sb", bufs=1))
        sb_x = ctx.enter_context(tc.tile_pool(name="sb_x", bufs=4))
        sb_w1 = ctx.enter_context(tc.tile_pool(name="sb_w1", bufs=3))
        sb_w2 = ctx.enter_context(tc.tile_pool(name="sb_w2", bufs=3))
        sb_e = ctx.enter_context(tc.tile_pool(name="sb_e", bufs=3))
        sb_o = ctx.enter_context(tc.tile_pool(name="sb_o", bufs=3))
        ps = ctx.enter_context(tc.tile_pool(name="ps", bufs=2, space="PSUM"))

        # weights first on the sync queue (needed by the first matmul);
        # other constants go via the gpsimd queue so they don't delay x.
        wqkv = sb.tile([128, 8, 384], F32R)
        nc.sync.dma_start(out=wqkv[:], in_=d_wqkv.rearrange("(n p) c -> p n c", p=128))
        vi_t = sb.tile([128, NT, 128], F32)
        nc.gpsimd.dma_start(out=vi_t[:], in_=d_vis.rearrange("(n p) c -> p n c", p=128))
        cc_t = sb.tile([128, NT, 64], F32)
        nc.gpsimd.dma_start(out=cc_t[:], in_=d_cc.rearrange("(n p) c -> p n c", p=128))
        sc_t = sb.tile([128, NT, 64], F32)
        nc.gpsimd.dma_start(out=sc_t[:], in_=d_sc.rearrange("(n p) c -> p n c", p=128))
        wo_h = [sb.tile([64, D], F32R, tag=f"wo{h}", name=f"wo{h}") for h in range(2)]
        nc.gpsimd.dma_start(out=wo_h[0][:], in_=d_wo0[:])
        nc.gpsimd.dma_start(out=wo_h[1][:], in_=d_wo1[:])
        idn = sb.tile([128, 128], F32R)
        nc.gpsimd.dma_start(out=idn[:], in_=d_idn[:])
        tri = sb.tile([128, 128], F32)
        nc.gpsimd.dma_start(out=tri[:], in_=d_tri[:])
        esk = sb.tile([1, 130], F32R)
        nc.gpsimd.dma_start(out=esk[:], in_=d_esk[:])
        on1 = sb.tile([1, 512], F32R)
        nc.gpsimd.dma_start(out=on1[:], in_=d_on1[:])
        onp = sb.tile([65, 64], F32R)
        nc.gpsimd.dma_start(out=onp[:], in_=d_onp[:])
        on2 = sb.tile([128, 2], F32)
        nc.gpsimd.dma_start(out=on2[:], in_=d_on2[:])

        stats = sb.tile([128, 64], F32)
        rbuf = sb.tile([128, 64], F32)
        qT = sb.tile([128, T], F32R)
        kT = sb.tile([128, T], F32R)
        vtiles = [sb.tile([128, 130], F32R, tag=f"v{i}", name=f"v{i}") for i in range(NT)]
        qkr = [sb.tile([128, 256], F32R, tag=f"qkr{i}", name=f"qkr{i}") for i in range(NT)]
        qkro = [sb.tile([128, 256], F32, tag=f"qkro{i}", name=f"qkro{i}") for i in range(NT)]
        lsb = sb.tile([65, 8, 512], F32R)
        yts = [sb.tile([64, T], F32R, tag=f"yts{h}", name=f"yts{h}") for h in range(2)]

        # ---------------- P1: fused QKV + stats + rope + v-blend ----------
        # PSUM tags (8 banks static): qkvtr 2x1 + st 2x2 + ytmo 2x1 = 8
        for ti in range(NT):
            xt = sb_x.tile([128, 8, 128], F32R, tag="xt", name=f"xt{ti}")
            nc.sync.dma_start(
                out=xt[:],
                in_=d_xT[:, 128 * ti : 128 * (ti + 1)].rearrange(
                    "(n p) c -> p n c", p=128
                ),
            )
            psq = ps.tile([128, 384], F32, tag="qkvtr", name=f"psq{ti}")
            for i in range(8):
                nc.tensor.matmul(
                    psq[:], xt[:, i, :], wqkv[:, i, :],
                    start=(i == 0), stop=(i == 7),
                )
            # single fast evacuation of the whole qkv tile (frees the bank)
            qsb = sb_w1.tile([128, 384], F32, tag="qsb", name=f"qsb{ti}")
            nc.scalar.copy(qsb[:], psq[:])
            # stats: Square (ACT) + segmented reduce (DVE), from SBUF
            sqt = sb_w1.tile([128, 256], F32, tag="sqt", name=f"sqt{ti}")
            nc.scalar.activation(sqt[:], qsb[:, 0:256], AF.Square)
            nc.vector.tensor_reduce(
                stats[:, 4 * ti : 4 * ti + 4],
                sqt[:].rearrange("p (s c) -> p s c", s=4),
                axis=AX.X, op=ALU.add,
            )
            # v-blend on GPSIMD (SBUF only) + ones cols
            vt = vtiles[ti]
            nc.gpsimd.tensor_tensor(
                out=vt[:].rearrange("p (s c) -> p s c", s=2)[:, :, 0:64],
                in0=qsb[:, 256:384].rearrange("p (s c) -> p s c", s=2),
                in1=vi_t[:, ti, :].rearrange("p (s c) -> p s c", s=2),
                op=ALU.add,
            )
            nc.vector.tensor_copy(
                vt[:].rearrange("p (s c) -> p s c", s=2)[:, :, 64:65],
                on2[:].rearrange("p (s c) -> p s c", s=2),
            )
            # rope: swap (GPSIMD) -> mult1 (DVE) / mult2 (GPSIMD) -> add (DVE)
            qksw = sb_w1.tile([128, 256], F32, tag="qksw", name=f"qksw{ti}")
            qk4 = qsb[:, 0:256].rearrange("p (s h c) -> p s h c", s=4, h=2)
            sw4 = qksw[:].rearrange("p (s h c) -> p s h c", s=4, h=2)
            nc.gpsimd.tensor_copy(sw4[:, :, 0, :], qk4[:, :, 1, :])
            nc.gpsimd.tensor_copy(sw4[:, :, 1, :], qk4[:, :, 0, :])
            tcos = sb_w2.tile([128, 256], F32, tag="tcos", name=f"tcos{ti}")
            nc.vector.tensor_tensor(
                out=tcos[:].rearrange("p (s c) -> p s c", s=4),
                in0=qsb[:, 0:256].rearrange("p (s c) -> p s c", s=4),
                in1=cc_t[:, ti, :].unsqueeze(1).broadcast_to((128, 4, 64)),
                op=ALU.mult,
            )
            tsin = sb_w2.tile([128, 256], F32, tag="tsin", name=f"tsin{ti}")
            nc.gpsimd.tensor_tensor(
                out=tsin[:].rearrange("p (s c) -> p s c", s=4),
                in0=qksw[:].rearrange("p (s c) -> p s c", s=4),
                in1=sc_t[:, ti, :].unsqueeze(1).broadcast_to((128, 4, 64)),
                op=ALU.mult,
            )
            nc.vector.tensor_tensor(
                out=qkro[ti][:], in0=tcos[:], in1=tsin[:], op=ALU.add
            )
            # batched rsqrt per group of 4 tiles (DVE bit-trick + 2 Newton
            # iters; keeps ACT on the exp table only), then apply + transpose
            if ti % 4 == 3:
                g = 16 * (ti // 4)
                rs = rbuf[:, g : g + 16]
                zt = sb_w2.tile([128, 16], F32, tag="zt", name=f"zt{ti}")
                nt1 = sb_w2.tile([128, 16], F32, tag="nt1", name=f"nt1{ti}")
                nc.vector.tensor_scalar(
                    out=zt[:], in0=stats[:, g : g + 16], scalar1=1.0 / 64.0,
                    scalar2=RMS_EPS, op0=ALU.mult, op1=ALU.add,
                )
                nc.vector.tensor_scalar(
                    out=nt1[:].bitcast(I32), in0=zt[:].bitcast(I32), scalar1=1,
                    scalar2=0xFFFFFFFF, op0=ALU.logical_shift_right,
                    op1=ALU.bitwise_xor,
                )
                nc.vector.tensor_scalar(
                    out=rs.bitcast(I32), in0=nt1[:].bitcast(I32),
                    scalar1=0x5F3759E0, scalar2=None, op0=ALU.add,
                )
                for _ in range(2):
                    nc.vector.tensor_tensor(out=nt1[:], in0=rs, in1=rs, op=ALU.mult)
                    nc.vector.tensor_tensor(out=nt1[:], in0=nt1[:], in1=zt[:], op=ALU.mult)
                    nc.vector.tensor_scalar(
                        out=nt1[:], in0=nt1[:], scalar1=-0.5, scalar2=1.5,
                        op0=ALU.mult, op1=ALU.add,
                    )
                    nc.vector.tensor_tensor(out=rs, in0=rs, in1=nt1[:], op=ALU.mult)
                for tj in range(ti - 3, ti + 1):
                    for s in range(4):  # qA qB kA kB (k gets 0.125 folded)
                        if s < 2:
                            nc.vector.tensor_scalar_mul(
                                qkr[tj][:, 64 * s : 64 * s + 64],
                                qkro[tj][:, 64 * s : 64 * s + 64],
                                rbuf[:, 4 * tj + s : 4 * tj + s + 1],
                            )
                        else:
                            nc.vector.tensor_scalar(
                                out=qkr[tj][:, 64 * s : 64 * s + 64],
                                in0=qkro[tj][:, 64 * s : 64 * s + 64],
                                scalar1=rbuf[:, 4 * tj + s : 4 * tj + s + 1],
                                scalar2=0.125, op0=ALU.mult, op1=ALU.mult,
                            )
                # P2: transposes for this group -> qT, kT
                for tj in range(ti - 3, ti + 1):
                    for which, dst in ((0, qT), (1, kT)):
                        ptr = ps.tile(
                            [128, 128], F32R, tag="qkvtr", name=f"tr{tj}_{which}"
                        )
                        nc.tensor.transpose(
                            ptr[:], qkr[tj][:, 128 * which : 128 * which + 128], idn[:]
                        )
                        nc.vector.tensor_copy(
                            dst[:, 128 * tj : 128 * (tj + 1)], ptr[:]
                        )

        # ---------------- P3 + P4: attention, scale, out-proj ------------
        for ci in range(4):
            yt_h = []
            for h in range(2):
                kj_max = 4 * ci + 4
                yt = ps.tile([128, 512], F32, tag="ytmo", name=f"yt{ci}_{h}")
                yt_h.append(yt)
                for dd in range(kj_max // 2):
                    st = ps.tile([128, 1024], F32, tag="st", name=f"st{ci}_{h}_{dd}")
                    et = sb_e.tile([128, 1024], F32R, tag="et", name=f"et{ci}_{h}_{dd}")
                    qs = {}
                    for j2 in range(2):
                        kj = 2 * dd + j2
                        qs[j2] = 128 * (kj - 4 * ci) if kj >= 4 * ci else 0
                        nc.tensor.matmul(
                            st[:, 512 * j2 + qs[j2] : 512 * (j2 + 1)],
                            kT[64 * h : 64 * h + 64, 128 * kj : 128 * (kj + 1)],
                            qT[64 * h : 64 * h + 64, 512 * ci + qs[j2] : 512 * (ci + 1)],
                            start=True, stop=True,
                        )
                    nc.scalar.activation(et[:], st[:], AF.Exp)
                    for j2 in range(2):
                        kj = 2 * dd + j2
                        if kj >= 4 * ci:  # diagonal: tri-mask the block
                            blk = et[:, 512 * j2 + qs[j2] : 512 * j2 + qs[j2] + 128]
                            nc.gpsimd.tensor_tensor(
                                out=blk, in0=blk.bitcast(F32), in1=tri[:], op=ALU.mult
                            )
                    for j2 in range(2):
                        kj = 2 * dd + j2
                        nc.tensor.matmul(
                            yt[0:65, qs[j2] : 512],
                            vtiles[kj][:, 65 * h : 65 * h + 65],
                            et[:, 512 * j2 + qs[j2] : 512 * (j2 + 1)],
                            start=(kj == 0), stop=False,
                        )
                # + e^sink into the L row (row 64)
                nc.tensor.matmul(
                    yt[0:65, 0:512],
                    esk[:, 65 * h : 65 * h + 65],
                    on1[:],
                    start=False, stop=True,
                )
            # P4 for this ci
            mb = ps.tile([128, 1024], F32, tag="st", name=f"mb{ci}")
            mbs = sb_w2.tile([64, 1024], F32, tag="mbs", name=f"mbs{ci}")
            for h in range(2):
                nc.vector.tensor_copy(
                    lsb[64:65, 4 * h + ci, :], yt_h[h][64:65, 0:512]
                )
                nc.tensor.matmul(
                    mb[0:64, 512 * h : 512 * h + 512],
                    onp[64:65, 0:64],
                    lsb[64:65, 4 * h + ci, :],
                    start=True, stop=True,
                )
                nc.vector.reciprocal_approx_fast(
                    out=mbs[0:64, 512 * h : 512 * h + 512],
                    in_=mb[0:64, 512 * h : 512 * h + 512],
                )
                nc.vector.tensor_tensor(
                    out=yts[h][0:64, 512 * ci : 512 * (ci + 1)],
                    in0=yt_h[h][0:64, 0:512],
                    in1=mbs[0:64, 512 * h : 512 * h + 512],
                    op=ALU.mult,
                )
            for tt in range(4 * ci, 4 * ci + 4):
                outsb = sb_o.tile([128, 1024], F32, tag="outsb", name=f"osb{tt}")
                for jc in range(2):
                    pso = ps.tile([128, 512], F32, tag="ytmo", name=f"pso{tt}_{jc}")
                    for h in range(2):
                        nc.tensor.matmul(
                            pso[:],
                            yts[h][0:64, 128 * tt : 128 * (tt + 1)],
                            wo_h[h][0:64, 512 * jc : 512 * (jc + 1)],
                            start=(h == 0), stop=(h == 1),
                        )
                    if jc == 0:
                        nc.vector.tensor_copy(outsb[:, 0:512], pso[:])
                    else:
                        nc.scalar.copy(outsb[:, 512:1024], pso[:])
                nc.sync.dma_start(
                    out=d_out[128 * tt : 128 * (tt + 1), :], in_=outsb[:]
                )

    nc.compile()
    return nc


_NC = None


def _rope_tables():
    inv = (1.0 / 10000.0) ** (np.arange(0, HD, 2, dtype=np.float64) / HD)
    t = np.arange(T, dtype=np.float64)
    f = np.outer(t, inv)  # (T, 32)
    cc = np.concatenate([np.cos(f), np.cos(f)], axis=1).astype(np.float32)
    sc = np.concatenate([np.sin(f), -np.sin(f)], axis=1).astype(np.float32)
    return cc, sc


def kernel(x, vi, Wq, Wk, Wv, Wo, lamb, sink_weights):
    global _NC
    x = np.asarray(x, dtype=np.float32)
    vi = np.asarray(vi, dtype=np.float32)
    Wq = np.asarray(Wq, dtype=np.float32)
    Wk = np.asarray(Wk, dtype=np.float32)
    Wv = np.asarray(Wv, dtype=np.float32)
    Wo = np.asarray(Wo, dtype=np.float32)
    lam = float(np.asarray(lamb).reshape(-1)[0])
    sink = np.asarray(sink_weights, dtype=np.float32).reshape(-1)

    if _NC is None:
        _NC = _build_program()

    xT = np.ascontiguousarray(x[0].T)  # (D, T)
    cc, sc = _rope_tables()
    tri = (np.arange(128)[None, :] >= np.arange(128)[:, None]).astype(np.float32)
    idn = np.eye(128, dtype=np.float32)
    on1 = np.ones((1, 512), np.float32)
    onp = np.zeros((65, 64), np.float32)
    onp[64, :] = 1.0
    on2 = np.ones((128, 2), np.float32)

    in_maps = []
    for c in range(8):
        lo = 128 * c
        wqkv = np.ascontiguousarray(
            np.concatenate(
                [
                    Wq[lo : lo + 128].T,
                    Wk[lo : lo + 128].T,
                    (1.0 - lam) * Wv[lo : lo + 128].T,
                ],
                axis=1,
            )
        )  # (D, 384)
        esk = np.zeros((1, 130), np.float32)
        esk[0, 64] = np.exp(sink[2 * c])
        esk[0, 129] = np.exp(sink[2 * c + 1])
        in_maps.append(
            {
                "xT": xT,
                "wqkv": wqkv,
                "vis": np.ascontiguousarray(lam * vi[0][:, lo : lo + 128]),
                "cc": cc,
                "sc": sc,
                "wo0": np.ascontiguousarray(Wo[:, lo : lo + 64].T),
                "wo1": np.ascontiguousarray(Wo[:, lo + 64 : lo + 128].T),
                "idn": idn,
                "tri": tri,
                "esk": esk,
                "on1": on1,
                "onp": onp,
                "on2": on2,
            }
        )

    global _trace_in_maps
    _trace_in_maps = in_maps
    res = run_bass_kernel_spmd(_NC, in_maps, list(range(8)))
    out = np.zeros((T, D), np.float64)
    for c in range(8):
        out += res.results[c]["out"]
    return out.astype(np.float32).reshape(1, T, D)
